# revision 1
# baseline (speedup 1.0000x reference)
"""Trainium2 Bass kernel for nn_NodeNet (GNN message passing).

Strategy: data-parallel over graphs across 8 NeuronCores. Host transposes
inputs into [feature, row] layouts so every DMA is contiguous; all matmuls
run in fp32r (full-rate fp32) with transposed activations:
  node stage: dp[128, rows] -> MLP -> sum over datapoints -> feature_enc[64, G]
  edge stage: rhs = [fe (broadcast per graph); edge_attr^T] -> MLP -> out^T
The structured fast path (edges grouped 128-per-graph, all within-graph, as
produced by the reference's setup_inputs) runs one fused launch per core with
feature_enc kept on-chip. A general fallback handles arbitrary edge_index /
batch with two launches and a host-side gather + mask.
"""

import os
import sys

import ml_dtypes
import numpy as np

BF16NP = ml_dtypes.bfloat16

if "/opt/trn_rl_repo" not in sys.path and os.path.isdir("/opt/trn_rl_repo"):
    sys.path.insert(0, "/opt/trn_rl_repo")

import concourse.bacc as bacc
import concourse.tile as tile
from concourse import mybir
from concourse.bass_utils import run_bass_kernel_spmd

G, ODE, NDATA, H, EA, EPG = 4096, 64, 32, 256, 64, 128
E = G * EPG
NCORES = 8
GC = G // NCORES           # graphs per core
RC = GC * NDATA            # node-MLP rows per core
EC = GC * EPG              # edges per core
TN = 512                   # tile free size
GT = TN // NDATA           # graphs covered per node tile (16)
GE = TN // EPG             # graphs covered per edge tile (4)

F32 = mybir.dt.float32
F32R = mybir.dt.float32r
BF16 = mybir.dt.bfloat16
RELU = mybir.ActivationFunctionType.Relu
IDENT = mybir.ActivationFunctionType.Identity
ADD = mybir.AluOpType.add
MAX = mybir.AluOpType.max
AXX = mybir.AxisListType.X

_PROGRAMS = {}
last_results = None


def _install_trace_shim():
    """Optional: make trace=True work by injecting antenv.axon_hooks."""
    import types

    if "antenv.axon_hooks" in sys.modules:
        return
    try:
        mod = types.ModuleType("antenv.axon_hooks")
        mod._hook = None
        mod.set_axon_ntff_profile_hook = lambda h: setattr(mod, "_hook", h)
        mod.get_axon_ntff_profile_hook = lambda: mod._hook
        sys.modules["antenv.axon_hooks"] = mod
        import antenv

        antenv.axon_hooks = mod
        from trn_agent_boot.trn_boot import _ntff_profile_via_ctypes

        hook = _ntff_profile_via_ctypes("/opt/axon/libaxon_pjrt.so")
        if hook is not None:
            mod.set_axon_ntff_profile_hook(hook)
    except Exception:
        pass


def _declare_weights(nc):
    t = {}
    t["nw1"] = nc.dram_tensor("nw1", [128, H], BF16, kind="ExternalInput")
    t["nw2"] = nc.dram_tensor("nw2", [128, 2, H], BF16, kind="ExternalInput")
    t["nw3"] = nc.dram_tensor("nw3", [128, 2, ODE], BF16, kind="ExternalInput")
    t["nb1"] = nc.dram_tensor("nb1", [128, 2], F32, kind="ExternalInput")
    t["nb2"] = nc.dram_tensor("nb2", [128, 2], F32, kind="ExternalInput")
    t["nb3"] = nc.dram_tensor("nb3", [ODE, 1], F32, kind="ExternalInput")
    t["ew1"] = nc.dram_tensor("ew1", [128, H], BF16, kind="ExternalInput")
    t["ew2"] = nc.dram_tensor("ew2", [128, 2, H], BF16, kind="ExternalInput")
    t["ew3"] = nc.dram_tensor("ew3", [128, 2, ODE], BF16, kind="ExternalInput")
    t["eb1"] = nc.dram_tensor("eb1", [128, 2], F32, kind="ExternalInput")
    t["eb2"] = nc.dram_tensor("eb2", [128, 2], F32, kind="ExternalInput")
    t["eb3"] = nc.dram_tensor("eb3", [EA, 1], F32, kind="ExternalInput")
    return t


def _load_weights(nc, consts, td, node: bool, edge: bool):
    sb = {}
    names = []
    if node:
        names += ["nw1", "nw2", "nw3", "nb1", "nb2", "nb3"]
    if edge:
        names += ["ew1", "ew2", "ew3", "eb1", "eb2", "eb3"]
    for n in names:
        d = td[n]
        sb[n] = consts.tile(list(d.shape), d.dtype, tag=n, name=n)
        nc.sync.dma_start(sb[n], d[:])
    return sb


def _emit_node_stage(nc, pools, w, xT_d, hsum):
    consts, xin, hid, ps1, ps2, ps3 = pools
    for p in range(RC // (2 * TN)):
        r0 = p * 2 * TN
        xtp = xin.tile([128, 2, TN], BF16, tag="xt")
        nc.sync.dma_start(xtp, xT_d[:, r0:r0 + 2 * TN].rearrange("c (t e) -> c t e", t=2))
        h1p = hid.tile([128, 2, 2, TN], BF16, tag="h1")
        for t01 in (0, 1):
            ps_a = ps1.tile([128, TN], F32, tag="l1a")
            ps_b = ps1.tile([128, TN], F32, tag="l1b")
            nc.tensor.matmul(ps_a, w["nw1"][:, 0:128], xtp[:, t01], start=True, stop=True)
            nc.tensor.matmul(ps_b, w["nw1"][:, 128:256], xtp[:, t01], start=True, stop=True)
            nc.scalar.activation(h1p[:, 0, t01], ps_a, RELU, bias=w["nb1"][:, 0:1])
            nc.vector.tensor_scalar(
                out=h1p[:, 1, t01], in0=ps_b, scalar1=w["nb1"][:, 1:2], scalar2=0.0,
                op0=ADD, op1=MAX,
            )
        l2ap = ps2.tile([128, 2, TN], F32, tag="l2a")
        l2bp = ps2.tile([128, 2, TN], F32, tag="l2b")
        for t01 in (0, 1):
            for k in (0, 1):
                nc.tensor.matmul(l2ap[:, t01], w["nw2"][:, k, 0:128], h1p[:, k, t01],
                                 start=(k == 0), stop=(k == 1))
            for k in (0, 1):
                nc.tensor.matmul(l2bp[:, t01], w["nw2"][:, k, 128:256], h1p[:, k, t01],
                                 start=(k == 0), stop=(k == 1))
        h2p = hid.tile([128, 2, 2, TN], BF16, tag="h2")
        nc.scalar.activation(h2p[:, 0], l2ap, RELU, bias=w["nb2"][:, 0:1])
        nc.vector.tensor_scalar(
            out=h2p[:, 1], in0=l2bp, scalar1=w["nb2"][:, 1:2], scalar2=0.0,
            op0=ADD, op1=MAX,
        )
        with nc.allow_low_precision(reason="bf16 reduce feeds bf16 matmul"):
            nc.vector.reduce_sum(
                out=hsum[:, :, p * 2 * GT:(p + 1) * 2 * GT],
                in_=h2p.rearrange("c k t (g d) -> c (k t g) d", d=NDATA),
                axis=AXX,
            )
    # feature_enc = hsum @ nw3 + nb3   -> [64, GC]
    ps_f = ps3.tile([ODE, 2, TN], F32, tag="l3")
    for k in (0, 1):
        nc.tensor.matmul(ps_f[:, 0], w["nw3"][:, k], hsum[:, k],
                         start=(k == 0), stop=(k == 1))
    return ps_f[:, 0]


def _emit_edge_stage(nc, pools, w, attrT_d, outT_d, fe_src):
    """fe_src: ("sbuf", feT_sb) or ("dram", feTg_d)."""
    consts, xin, hid, ps1, ps2, ps3 = pools
    for p in range(EC // (2 * TN)):
        e0 = p * 2 * TN
        g0 = p * 2 * GE
        rtp = xin.tile([128, 2, TN], BF16, tag="rt")
        nc.sync.dma_start(rtp[64:128],
                          attrT_d[:, e0:e0 + 2 * TN].rearrange("c (t e) -> c t e", t=2))
        if fe_src[0] == "sbuf":
            feT_sb = fe_src[1]
            nc.vector.tensor_copy(
                out=rtp[0:64].rearrange("c t (g e) -> c (t g) e", e=EPG),
                in_=feT_sb[:, g0:g0 + 2 * GE, None].to_broadcast([ODE, 2 * GE, EPG]),
            )
        else:
            nc.sync.dma_start(rtp[0:64],
                              fe_src[1][:, e0:e0 + 2 * TN].rearrange("c (t e) -> c t e", t=2))
        e1p = hid.tile([128, 2, 2, TN], BF16, tag="h1")
        for t01 in (0, 1):
            ps_a = ps1.tile([128, TN], F32, tag="l1a")
            ps_b = ps1.tile([128, TN], F32, tag="l1b")
            nc.tensor.matmul(ps_a, w["ew1"][:, 0:128], rtp[:, t01], start=True, stop=True)
            nc.tensor.matmul(ps_b, w["ew1"][:, 128:256], rtp[:, t01], start=True, stop=True)
            nc.scalar.activation(e1p[:, 0, t01], ps_a, RELU, bias=w["eb1"][:, 0:1])
            nc.vector.tensor_scalar(
                out=e1p[:, 1, t01], in0=ps_b, scalar1=w["eb1"][:, 1:2], scalar2=0.0,
                op0=ADD, op1=MAX,
            )
        l2ap = ps2.tile([128, 2, TN], F32, tag="l2a")
        l2bp = ps2.tile([128, 2, TN], F32, tag="l2b")
        for t01 in (0, 1):
            for k in (0, 1):
                nc.tensor.matmul(l2ap[:, t01], w["ew2"][:, k, 0:128], e1p[:, k, t01],
                                 start=(k == 0), stop=(k == 1))
            for k in (0, 1):
                nc.tensor.matmul(l2bp[:, t01], w["ew2"][:, k, 128:256], e1p[:, k, t01],
                                 start=(k == 0), stop=(k == 1))
        e2p = hid.tile([128, 2, 2, TN], BF16, tag="h2")
        nc.scalar.activation(e2p[:, 0], l2ap, RELU, bias=w["eb2"][:, 0:1])
        nc.vector.tensor_scalar(
            out=e2p[:, 1], in0=l2bp, scalar1=w["eb2"][:, 1:2], scalar2=0.0,
            op0=ADD, op1=MAX,
        )
        l3p = ps3.tile([ODE, 2, TN], F32, tag="l3")
        for t01 in (0, 1):
            for k in (0, 1):
                nc.tensor.matmul(l3p[:, t01], w["ew3"][:, k], e2p[:, k, t01],
                                 start=(k == 0), stop=(k == 1))
        otp = hid.tile([ODE, 2, TN], F32, tag="ot")
        if p % 2 == 0:
            nc.scalar.activation(otp, l3p, IDENT, bias=w["eb3"])
        else:
            nc.vector.tensor_scalar(out=otp, in0=l3p, scalar1=w["eb3"],
                                    scalar2=0.0, op0=ADD, op1=mybir.AluOpType.bypass)
        nc.sync.dma_start(outT_d[:, e0:e0 + 2 * TN],
                          otp.rearrange("c t e -> c (t e)"))


def _build(mode):
    """mode: 'fused' (node+edge, fe on-chip), 'node', 'edge'."""
    nc = bacc.Bacc("TRN2", target_bir_lowering=False)
    td = _declare_weights(nc)
    if mode in ("fused", "node"):
        xT_d = nc.dram_tensor("xT", [128, RC], BF16, kind="ExternalInput")
    if mode in ("fused", "edge"):
        attrT_d = nc.dram_tensor("attrT", [64, EC], BF16, kind="ExternalInput")
        outT_d = nc.dram_tensor("outT", [64, EC], F32, kind="ExternalOutput")
    if mode == "edge":
        feTg_d = nc.dram_tensor("feTg", [64, EC], BF16, kind="ExternalInput")
    if mode == "node":
        feT_out = nc.dram_tensor("feT", [ODE, GC], F32, kind="ExternalOutput")

    with tile.TileContext(nc) as tc:
        with (
            tc.tile_pool(name="consts", bufs=1) as consts,
            tc.tile_pool(name="xin", bufs=4) as xin,
            tc.tile_pool(name="hid", bufs=3) as hid,
            tc.tile_pool(name="ps1", bufs=1, space="PSUM") as ps1,
            tc.tile_pool(name="ps2", bufs=1, space="PSUM") as ps2,
            tc.tile_pool(name="ps3", bufs=1, space="PSUM") as ps3,
        ):
            pools = (consts, xin, hid, ps1, ps2, ps3)
            w = _load_weights(nc, consts, td,
                              node=mode in ("fused", "node"),
                              edge=mode in ("fused", "edge"))
            if mode in ("fused", "node"):
                hsum = consts.tile([128, 2, GC], BF16, tag="hsum")
                ps_f = _emit_node_stage(nc, pools, w, xT_d, hsum)
            if mode == "fused":
                feT_sb = consts.tile([ODE, GC], BF16, tag="feT")
                nc.scalar.activation(feT_sb, ps_f, IDENT, bias=w["nb3"])
                _emit_edge_stage(nc, pools, w, attrT_d, outT_d, ("sbuf", feT_sb))
            elif mode == "node":
                feT_sb = consts.tile([ODE, GC], F32, tag="feT")
                nc.scalar.activation(feT_sb, ps_f, IDENT, bias=w["nb3"])
                nc.sync.dma_start(feT_out[:], feT_sb)
            elif mode == "edge":
                _emit_edge_stage(nc, pools, w, attrT_d, outT_d, ("dram", feTg_d))
    nc.finalize()
    return nc


def _get_program(mode):
    if mode not in _PROGRAMS:
        _PROGRAMS[mode] = _build(mode)
    return _PROGRAMS[mode]


def _shared_weight_arrays(kw):
    f = np.float32
    c = np.ascontiguousarray
    return {
        "nw1": c(np.asarray(kw["node_w1"], dtype=f).astype(BF16NP)),
        "nw2": c(np.asarray(kw["node_w2"], dtype=f).reshape(2, 128, H).transpose(1, 0, 2).astype(BF16NP)),
        "nw3": c(np.asarray(kw["node_w3"], dtype=f).reshape(2, 128, ODE).transpose(1, 0, 2).astype(BF16NP)),
        "nb1": c(np.asarray(kw["node_b1"], dtype=f).reshape(2, 128).T),
        "nb2": c(np.asarray(kw["node_b2"], dtype=f).reshape(2, 128).T),
        "nb3": c(np.asarray(kw["node_b3"], dtype=f).reshape(ODE, 1)),
        "ew1": c(np.asarray(kw["edge_w1"], dtype=f).astype(BF16NP)),
        "ew2": c(np.asarray(kw["edge_w2"], dtype=f).reshape(2, 128, H).transpose(1, 0, 2).astype(BF16NP)),
        "ew3": c(np.asarray(kw["edge_w3"], dtype=f).reshape(2, 128, ODE).transpose(1, 0, 2).astype(BF16NP)),
        "eb1": c(np.asarray(kw["edge_b1"], dtype=f).reshape(2, 128).T),
        "eb2": c(np.asarray(kw["edge_b2"], dtype=f).reshape(2, 128).T),
        "eb3": c(np.asarray(kw["edge_b3"], dtype=f).reshape(EA, 1)),
    }


def _x_transposed_per_core(x, c):
    xs = np.asarray(x, dtype=np.float32).reshape(G, ODE, 2, NDATA)[c * GC:(c + 1) * GC]
    return np.ascontiguousarray(xs.transpose(1, 2, 0, 3).reshape(128, RC).astype(BF16NP))


def kernel(x, edge_attr, node_w1, node_b1, node_w2, node_b2, node_w3, node_b3,
           edge_w1, edge_b1, edge_w2, edge_b2, edge_w3, edge_b3,
           edge_index, batch):
    global last_results
    kw = dict(x=x, node_w1=node_w1, node_b1=node_b1, node_w2=node_w2,
              node_b2=node_b2, node_w3=node_w3, node_b3=node_b3,
              edge_w1=edge_w1, edge_b1=edge_b1, edge_w2=edge_w2,
              edge_b2=edge_b2, edge_w3=edge_w3, edge_b3=edge_b3)
    trace = os.environ.get("KERNEL_TRACE", "") == "1"
    if trace:
        _install_trace_shim()

    edge_attr = np.asarray(edge_attr, dtype=np.float32)
    ei = np.asarray(edge_index)
    bt = np.asarray(batch)
    g_src = bt[ei[0]]
    g_dst = bt[ei[1]]
    same = g_src == g_dst
    structured = bool((g_src == np.repeat(np.arange(G), EPG)).all())

    shared = _shared_weight_arrays(kw)
    run_kwargs = dict(core_ids=list(range(NCORES)), trace=trace,
                      trace_cores=[0] if trace else None)

    if structured:
        nc = _get_program("fused")
        in_maps = []
        for c in range(NCORES):
            m = dict(shared)
            m["xT"] = _x_transposed_per_core(x, c)
            m["attrT"] = np.ascontiguousarray(edge_attr[c * EC:(c + 1) * EC].T.astype(BF16NP))
            in_maps.append(m)
        res = run_bass_kernel_spmd(nc, in_maps, **run_kwargs)
        last_results = res
        out = np.empty((E, EA), dtype=np.float32)
        for c in range(NCORES):
            out[c * EC:(c + 1) * EC] = res.results[c]["outT"].T
    else:
        # general path: node stage -> host gather of feature_enc -> edge stage
        nc_node = _get_program("node")
        in_maps = []
        for c in range(NCORES):
            m = dict(shared)
            m["xT"] = _x_transposed_per_core(x, c)
            in_maps.append(m)
        res_n = run_bass_kernel_spmd(nc_node, in_maps, **run_kwargs)
        feT_full = np.concatenate([res_n.results[c]["feT"] for c in range(NCORES)],
                                  axis=1)          # [64, G]
        feTg = feT_full[:, g_src]                   # [64, E]
        nc_edge = _get_program("edge")
        in_maps = []
        for c in range(NCORES):
            m = dict(shared)
            m["attrT"] = np.ascontiguousarray(edge_attr[c * EC:(c + 1) * EC].T.astype(BF16NP))
            m["feTg"] = np.ascontiguousarray(feTg[:, c * EC:(c + 1) * EC].astype(BF16NP))
            in_maps.append(m)
        res = run_bass_kernel_spmd(nc_edge, in_maps, **run_kwargs)
        last_results = res
        out = np.empty((E, EA), dtype=np.float32)
        for c in range(NCORES):
            out[c * EC:(c + 1) * EC] = res.results[c]["outT"].T

    if not same.all():
        out = np.where(same[:, None], out, edge_attr)
    return out



# revision 3
# speedup vs baseline: 1.1960x; 1.1960x over previous
"""Trainium2 Bass kernel for nn_NodeNet (GNN message passing).

Strategy: data-parallel over graphs across 8 NeuronCores. Host transposes
inputs into [feature, row] layouts so every DMA is contiguous; all matmuls
bf16 (fp8 DoubleRow tested: h1 quantization alone exceeds the error budget).
Software-pipelined per-512-column tile loop: front stage (DMA + l1 + bias/relu
into bf16 h1) for tile t is emitted alongside the back stage (l2 + relu + l3 +
bias) for tile t-1, so the PE never waits on the pointwise engines. l3 outputs
of adjacent tiles are packed into one [128,512] PSUM bank via column
tile_position, halving the output-bias pointwise work. The node-stage
datapoint reduction runs as a tensor_tensor halving tree (2x DVE mode) instead
of 1x tensor_reduce. PSUM->SBUF pointwise is split Scalar/Vector by the errata
cost model. Structured fast path (edges grouped 128-per-graph) runs one fused
launch per core; a general fallback handles arbitrary edge_index / batch.
"""

import os
import sys

import ml_dtypes
import numpy as np

BF16NP = ml_dtypes.bfloat16

if "/opt/trn_rl_repo" not in sys.path and os.path.isdir("/opt/trn_rl_repo"):
    sys.path.insert(0, "/opt/trn_rl_repo")

import concourse.bacc as bacc
import concourse.tile as tile
from concourse import mybir
from concourse.bass_utils import run_bass_kernel_spmd

G, ODE, NDATA, H, EA, EPG = 4096, 64, 32, 256, 64, 128
E = G * EPG
NCORES = 8
GC = G // NCORES           # graphs per core (512)
RC = GC * NDATA            # node-MLP rows per core (16384)
EC = GC * EPG              # edges per core (65536)
TN = 512                   # tile free size
NT = RC // TN              # node tiles (32)
ET = EC // TN              # edge tiles (128)
GPT = TN // EPG            # graphs per edge tile (4)

F32 = mybir.dt.float32
BF16 = mybir.dt.bfloat16
RELU = mybir.ActivationFunctionType.Relu
IDENT = mybir.ActivationFunctionType.Identity
ADD = mybir.AluOpType.add
MAX = mybir.AluOpType.max

_PROGRAMS = {}
last_results = None


def _install_trace_shim():
    """Optional: make trace=True work by injecting antenv.axon_hooks."""
    import types

    if "antenv.axon_hooks" in sys.modules:
        return
    try:
        mod = types.ModuleType("antenv.axon_hooks")
        mod._hook = None
        mod.set_axon_ntff_profile_hook = lambda h: setattr(mod, "_hook", h)
        mod.get_axon_ntff_profile_hook = lambda: mod._hook
        sys.modules["antenv.axon_hooks"] = mod
        import antenv

        antenv.axon_hooks = mod
        from trn_agent_boot.trn_boot import _ntff_profile_via_ctypes

        hook = _ntff_profile_via_ctypes("/opt/axon/libaxon_pjrt.so")
        if hook is not None:
            mod.set_axon_ntff_profile_hook(hook)
    except Exception:
        pass


def _declare_weights(nc):
    t = {}
    t["nw1"] = nc.dram_tensor("nw1", [128, H], BF16, kind="ExternalInput")
    t["nw2"] = nc.dram_tensor("nw2", [128, 2, H], BF16, kind="ExternalInput")
    t["nw3"] = nc.dram_tensor("nw3", [128, 2, ODE], BF16, kind="ExternalInput")
    t["nb1"] = nc.dram_tensor("nb1", [128, 2], F32, kind="ExternalInput")
    t["nb2"] = nc.dram_tensor("nb2", [128, 2], F32, kind="ExternalInput")
    t["nb3"] = nc.dram_tensor("nb3", [ODE, 1], F32, kind="ExternalInput")
    t["ew1"] = nc.dram_tensor("ew1", [128, H], BF16, kind="ExternalInput")
    t["ew2"] = nc.dram_tensor("ew2", [128, 2, H], BF16, kind="ExternalInput")
    t["ew3"] = nc.dram_tensor("ew3", [128, 2, ODE], BF16, kind="ExternalInput")
    t["eb1"] = nc.dram_tensor("eb1", [128, 2], F32, kind="ExternalInput")
    t["eb2"] = nc.dram_tensor("eb2", [128, 2], F32, kind="ExternalInput")
    t["eb3"] = nc.dram_tensor("eb3", [128, 1], F32, kind="ExternalInput")
    return t


def _load_weights(nc, consts, td, node: bool, edge: bool):
    sb = {}
    names = []
    if node:
        names += ["nw1", "nw2", "nw3", "nb1", "nb2", "nb3"]
    if edge:
        names += ["ew1", "ew2", "ew3", "eb1", "eb2", "eb3"]
    for n in names:
        d = td[n]
        sb[n] = consts.tile(list(d.shape), d.dtype, tag=n, name=n)
        nc.sync.dma_start(sb[n], d[:])
    return sb


def _emit_node_stage(nc, pools, w, xT_d, hsum):
    """Software-pipelined: front(t) = dma+l1+h1, back(t-1) = l2+h2+tree."""
    consts, xin, hid, ps1, ps2, ps3 = pools
    prev = None
    for t in range(NT + 1):
        cur = None
        if t < NT:
            xtp = xin.tile([128, TN], BF16, tag="nxt")
            nc.sync.dma_start(xtp, xT_d[:, t * TN:(t + 1) * TN])
            pa = ps1.tile([128, TN], F32, tag="l1a")
            pb = ps1.tile([128, TN], F32, tag="l1b")
            nc.tensor.matmul(pa, w["nw1"][:, 0:128], xtp, start=True, stop=True)
            nc.tensor.matmul(pb, w["nw1"][:, 128:256], xtp, start=True, stop=True)
            h1p = hid.tile([128, 2, TN], BF16, tag="nh1")
            nc.scalar.activation(h1p[:, 0], pa, RELU, bias=w["nb1"][:, 0:1])
            nc.vector.tensor_scalar(out=h1p[:, 1], in0=pb, scalar1=w["nb1"][:, 1:2],
                                    scalar2=0.0, op0=ADD, op1=MAX)
            cur = (t, h1p)
        if prev is not None:
            tp, h1p_p = prev
            pa = ps2.tile([128, TN], F32, tag="l2a")
            pb = ps2.tile([128, TN], F32, tag="l2b")
            for k in (0, 1):
                nc.tensor.matmul(pa, w["nw2"][:, k, 0:128], h1p_p[:, k],
                                 start=(k == 0), stop=(k == 1))
            for k in (0, 1):
                nc.tensor.matmul(pb, w["nw2"][:, k, 128:256], h1p_p[:, k],
                                 start=(k == 0), stop=(k == 1))
            h2p = hid.tile([128, 2, TN], BF16, tag="nh2")
            nc.scalar.activation(h2p[:, 0], pa, RELU, bias=w["nb2"][:, 0:1])
            nc.scalar.activation(h2p[:, 1], pb, RELU, bias=w["nb2"][:, 1:2])
            # sum over NDATA=32 datapoints of each of the 16 graphs: halving tree
            with nc.allow_low_precision(reason="bf16 tree reduce feeds bf16 matmul"):
                v0 = h2p.rearrange("c k (g d) -> c k g d", d=NDATA)
                t1 = hid.tile([128, 2, 16, 16], BF16, tag="tr1")
                nc.vector.tensor_tensor(out=t1, in0=v0[:, :, :, 0:16], in1=v0[:, :, :, 16:32], op=ADD)
                t2 = hid.tile([128, 2, 16, 8], BF16, tag="tr2")
                nc.vector.tensor_tensor(out=t2, in0=t1[:, :, :, 0:8], in1=t1[:, :, :, 8:16], op=ADD)
                t3 = hid.tile([128, 2, 16, 4], BF16, tag="tr3")
                nc.vector.tensor_tensor(out=t3, in0=t2[:, :, :, 0:4], in1=t2[:, :, :, 4:8], op=ADD)
                t4 = hid.tile([128, 2, 16, 2], BF16, tag="tr4")
                nc.vector.tensor_tensor(out=t4, in0=t3[:, :, :, 0:2], in1=t3[:, :, :, 2:4], op=ADD)
                nc.vector.tensor_tensor(out=hsum[:, :, tp], in0=t4[:, :, :, 0],
                                        in1=t4[:, :, :, 1], op=ADD)
        prev = cur
    # feature_enc = hsum @ nw3 + nb3   -> [64, GC]
    ps_f = ps3.tile([128, TN], F32, tag="l3")
    hs = hsum.rearrange("c k t g -> c k (t g)")
    for k in (0, 1):
        nc.tensor.matmul(ps_f[0:64], w["nw3"][:, k], hs[:, k],
                         start=(k == 0), stop=(k == 1), tile_position=(0, 0))
    return ps_f


def _emit_edge_stage(nc, pools, w, attrT_d, outT_d, fe_src):
    """fe_src: ("sbuf", fe4T_sb) with x4-duplicated columns, or ("dram", feTg_d).

    Pipelined: front(t) = dma+bcast+l1+h1; back(t-1) = l2+h2; l3/e3/out are
    emitted for (t-2, t-1) pairs at odd t-1, packed into one [128,TN] PSUM bank.
    """
    consts, xin, hid, ps1, ps2, ps3 = pools
    prev = None    # (t, h1p)
    pend = []      # [(t, h2p), ...] tiles awaiting l3
    for t in range(ET + 2):
        cur = None
        if t < ET:
            e0 = t * TN
            g0 = t * GPT
            rtp = xin.tile([128, TN], BF16, tag="ert")
            nc.sync.dma_start(rtp[64:128], attrT_d[:, e0:e0 + TN])
            if fe_src[0] == "sbuf":
                fe4T = fe_src[1]
                # rtp[0:64] cols: e = glocal*128 + r*4 + f
                nc.vector.tensor_copy(
                    out=rtp[0:64].rearrange("c (g r f) -> c g r f", g=GPT, f=4),
                    in_=fe4T[:, g0:g0 + GPT, None, :].to_broadcast([ODE, GPT, EPG // 4, 4]),
                )
            else:
                nc.sync.dma_start(rtp[0:64], fe_src[1][:, e0:e0 + TN])
            pa = ps1.tile([128, TN], F32, tag="l1a")
            pb = ps1.tile([128, TN], F32, tag="l1b")
            nc.tensor.matmul(pa, w["ew1"][:, 0:128], rtp, start=True, stop=True)
            nc.tensor.matmul(pb, w["ew1"][:, 128:256], rtp, start=True, stop=True)
            h1p = hid.tile([128, 2, TN], BF16, tag="eh1")
            nc.scalar.activation(h1p[:, 0], pa, RELU, bias=w["eb1"][:, 0:1])
            nc.vector.tensor_scalar(out=h1p[:, 1], in0=pb, scalar1=w["eb1"][:, 1:2],
                                    scalar2=0.0, op0=ADD, op1=MAX)
            cur = (t, h1p)
        if prev is not None:
            tp, h1p_p = prev
            pa = ps2.tile([128, TN], F32, tag="l2a")
            pb = ps2.tile([128, TN], F32, tag="l2b")
            for k in (0, 1):
                nc.tensor.matmul(pa, w["ew2"][:, k, 0:128], h1p_p[:, k],
                                 start=(k == 0), stop=(k == 1))
            for k in (0, 1):
                nc.tensor.matmul(pb, w["ew2"][:, k, 128:256], h1p_p[:, k],
                                 start=(k == 0), stop=(k == 1))
            h2p = hid.tile([128, 2, TN], BF16, tag="eh2")
            nc.scalar.activation(h2p[:, 0], pa, RELU, bias=w["eb2"][:, 0:1])
            nc.vector.tensor_scalar(out=h2p[:, 1], in0=pb, scalar1=w["eb2"][:, 1:2],
                                    scalar2=0.0, op0=ADD, op1=MAX)
            pend.append((tp, h2p))
        if len(pend) == 2:
            # l3 for both pending tiles, col-tiled into one PSUM bank:
            # first tile -> partitions 0-63, second -> 64-127. k-outer order
            # so the two column groups run concurrently in the PE array.
            l3p = ps3.tile([128, TN], F32, tag="l3")
            for k in (0, 1):
                for j, (tj, h2j) in enumerate(pend):
                    nc.tensor.matmul(l3p[64 * j:64 * j + 64], w["ew3"][:, k], h2j[:, k],
                                     start=(k == 0), stop=(k == 1),
                                     tile_position=(0, 64 * j))
            otp = hid.tile([128, TN], F32, tag="eot")
            nc.scalar.activation(otp, l3p, IDENT, bias=w["eb3"])
            e0a = pend[0][0] * TN
            e0b = pend[1][0] * TN
            nc.sync.dma_start(outT_d[:, e0a:e0a + TN], otp[0:64])
            nc.sync.dma_start(outT_d[:, e0b:e0b + TN], otp[64:128])
            pend = []
        prev = cur


def _build(mode):
    """mode: 'fused' (node+edge, fe on-chip), 'node', 'edge'."""
    nc = bacc.Bacc("TRN2", target_bir_lowering=False)
    td = _declare_weights(nc)
    if mode in ("fused", "node"):
        xT_d = nc.dram_tensor("xT", [128, RC], BF16, kind="ExternalInput")
    if mode in ("fused", "edge"):
        attrT_d = nc.dram_tensor("attrT", [64, EC], BF16, kind="ExternalInput")
        outT_d = nc.dram_tensor("outT", [64, EC], F32, kind="ExternalOutput")
    if mode == "edge":
        feTg_d = nc.dram_tensor("feTg", [64, EC], BF16, kind="ExternalInput")
    if mode == "node":
        feT_out = nc.dram_tensor("feT", [ODE, GC], F32, kind="ExternalOutput")

    with tile.TileContext(nc) as tc:
        with (
            tc.tile_pool(name="consts", bufs=1) as consts,
            tc.tile_pool(name="xin", bufs=4) as xin,
            tc.tile_pool(name="hid", bufs=3) as hid,
            tc.tile_pool(name="ps1", bufs=2, space="PSUM") as ps1,
            tc.tile_pool(name="ps2", bufs=1, space="PSUM") as ps2,
            tc.tile_pool(name="ps3", bufs=2, space="PSUM") as ps3,
        ):
            pools = (consts, xin, hid, ps1, ps2, ps3)
            w = _load_weights(nc, consts, td,
                              node=mode in ("fused", "node"),
                              edge=mode in ("fused", "edge"))
            if mode in ("fused", "node"):
                hsum = consts.tile([128, 2, NT, 16], BF16, tag="hsum")
                ps_f = _emit_node_stage(nc, pools, w, xT_d, hsum)
            if mode == "fused":
                feT_sb = consts.tile([ODE, GC], BF16, tag="feT")
                nc.scalar.activation(feT_sb, ps_f[0:64], IDENT, bias=w["nb3"])
                fe4T = consts.tile([ODE, GC, 4], BF16, tag="fe4T")
                nc.vector.tensor_copy(
                    out=fe4T, in_=feT_sb[:, :, None].to_broadcast([ODE, GC, 4]))
                _emit_edge_stage(nc, pools, w, attrT_d, outT_d, ("sbuf", fe4T))
            elif mode == "node":
                feTf = consts.tile([ODE, GC], F32, tag="feTf")
                nc.scalar.activation(feTf, ps_f[0:64], IDENT, bias=w["nb3"])
                nc.sync.dma_start(feT_out[:], feTf)
            elif mode == "edge":
                _emit_edge_stage(nc, pools, w, attrT_d, outT_d, ("dram", feTg_d))
    nc.finalize()
    return nc


def _get_program(mode):
    if mode not in _PROGRAMS:
        _PROGRAMS[mode] = _build(mode)
    return _PROGRAMS[mode]


def _shared_weight_arrays(kw):
    f = np.float32
    c = np.ascontiguousarray
    b3 = np.asarray(kw["edge_b3"], dtype=f)
    return {
        "nw1": c(np.asarray(kw["node_w1"], dtype=f).astype(BF16NP)),
        "nw2": c(np.asarray(kw["node_w2"], dtype=f).reshape(2, 128, H).transpose(1, 0, 2).astype(BF16NP)),
        "nw3": c(np.asarray(kw["node_w3"], dtype=f).reshape(2, 128, ODE).transpose(1, 0, 2).astype(BF16NP)),
        "nb1": c(np.asarray(kw["node_b1"], dtype=f).reshape(2, 128).T),
        "nb2": c(np.asarray(kw["node_b2"], dtype=f).reshape(2, 128).T),
        "nb3": c(np.asarray(kw["node_b3"], dtype=f).reshape(ODE, 1)),
        "ew1": c(np.asarray(kw["edge_w1"], dtype=f).astype(BF16NP)),
        "ew2": c(np.asarray(kw["edge_w2"], dtype=f).reshape(2, 128, H).transpose(1, 0, 2).astype(BF16NP)),
        "ew3": c(np.asarray(kw["edge_w3"], dtype=f).reshape(2, 128, ODE).transpose(1, 0, 2).astype(BF16NP)),
        "eb1": c(np.asarray(kw["edge_b1"], dtype=f).reshape(2, 128).T),
        "eb2": c(np.asarray(kw["edge_b2"], dtype=f).reshape(2, 128).T),
        "eb3": c(np.concatenate([b3, b3]).reshape(128, 1)),
    }


def _x_transposed_per_core(x, c):
    xs = np.asarray(x, dtype=np.float32).reshape(G, ODE, 2, NDATA)[c * GC:(c + 1) * GC]
    return np.ascontiguousarray(xs.transpose(1, 2, 0, 3).reshape(128, RC).astype(BF16NP))


def kernel(x, edge_attr, node_w1, node_b1, node_w2, node_b2, node_w3, node_b3,
           edge_w1, edge_b1, edge_w2, edge_b2, edge_w3, edge_b3,
           edge_index, batch):
    global last_results
    kw = dict(x=x, node_w1=node_w1, node_b1=node_b1, node_w2=node_w2,
              node_b2=node_b2, node_w3=node_w3, node_b3=node_b3,
              edge_w1=edge_w1, edge_b1=edge_b1, edge_w2=edge_w2,
              edge_b2=edge_b2, edge_w3=edge_w3, edge_b3=edge_b3)
    trace = os.environ.get("KERNEL_TRACE", "") == "1"
    if trace:
        _install_trace_shim()

    edge_attr = np.asarray(edge_attr, dtype=np.float32)
    ei = np.asarray(edge_index)
    bt = np.asarray(batch)
    g_src = bt[ei[0]]
    g_dst = bt[ei[1]]
    same = g_src == g_dst
    structured = bool((g_src == np.repeat(np.arange(G), EPG)).all())

    shared = _shared_weight_arrays(kw)
    run_kwargs = dict(core_ids=list(range(NCORES)), trace=trace,
                      trace_cores=[0] if trace else None)

    if structured:
        nc = _get_program("fused")
        in_maps = []
        for c in range(NCORES):
            m = dict(shared)
            m["xT"] = _x_transposed_per_core(x, c)
            m["attrT"] = np.ascontiguousarray(edge_attr[c * EC:(c + 1) * EC].T.astype(BF16NP))
            in_maps.append(m)
        res = run_bass_kernel_spmd(nc, in_maps, **run_kwargs)
        last_results = res
        out = np.empty((E, EA), dtype=np.float32)
        for c in range(NCORES):
            out[c * EC:(c + 1) * EC] = res.results[c]["outT"].T
    else:
        # general path: node stage -> host gather of feature_enc -> edge stage
        nc_node = _get_program("node")
        in_maps = []
        for c in range(NCORES):
            m = dict(shared)
            m["xT"] = _x_transposed_per_core(x, c)
            in_maps.append(m)
        res_n = run_bass_kernel_spmd(nc_node, in_maps, **run_kwargs)
        feT_full = np.concatenate([res_n.results[c]["feT"] for c in range(NCORES)],
                                  axis=1)          # [64, G]
        feTg = feT_full[:, g_src]                   # [64, E]
        nc_edge = _get_program("edge")
        in_maps = []
        for c in range(NCORES):
            m = dict(shared)
            m["attrT"] = np.ascontiguousarray(edge_attr[c * EC:(c + 1) * EC].T.astype(BF16NP))
            m["feTg"] = np.ascontiguousarray(feTg[:, c * EC:(c + 1) * EC].astype(BF16NP))
            in_maps.append(m)
        res = run_bass_kernel_spmd(nc_edge, in_maps, **run_kwargs)
        last_results = res
        out = np.empty((E, EA), dtype=np.float32)
        for c in range(NCORES):
            out[c * EC:(c + 1) * EC] = res.results[c]["outT"].T
    if not same.all():
        out = np.where(same[:, None], out, edge_attr)
    return out
